# revision 6
# baseline (speedup 1.0000x reference)
"""MoE router (AutonomousRouter) for TRN2, 8 NeuronCores.

Computes reference:
    act    = einsum('bnd,edc->bnec', x, W)          B,N,D,E,C = 4,2048,2048,8,512
    logits = ||act||_2 over c                       [B,N,E]
    probs  = softmax(logits, -1)
    top-2 routing with capacity 640 (priority = order within k-major (choice, token) sequence)
    out    = stack([dispatch, combine])             [2,B,N,E,640] fp32

Sharding: data-parallel over tokens; core i <- tokens [i*1024, (i+1)*1024) of the
flattened [8192] token axis (= batch b=i//2, half i%2). Weights replicated.

Error budget: the graded metric is absmax/scale with scale ~1 (dispatch ones), so
routing DECISIONS (ordered top-2 per token) must match the fp32 reference exactly,
while combine probabilities only need ~2e-2 absolute. A single fp16 matmul pass
(11-bit mantissa, exact products accumulated in fp32 PSUM) gives logits to ~1.6e-3
absolute — enough to decide all tokens whose top-3 margin exceeds THETA, and to get
probs to ~3e-4. The few hundred tokens with margin < THETA are re-decided on host in
fp64 during the inter-phase glue (the same place the cross-core cumsum offsets are
combined), with near-exact ties (< TIE_TOL, i.e. below fp32 reference resolution)
broken toward the lower expert index. This replaces the previous 3x bf16-split
matmul (hh+hl+lh) with a single fp16 pass -- 3x less PE work.

Phase A (device): fp16 matmul -> square -> reduce = sum-of-squares logits proxy.
Host glue: logits/softmax/top-2 + fp64 refinement of near-tie tokens + exact
integer capacity cumsums + cross-core offsets.
Phase B (device): per-(token,choice) one-hot rows (iota==slot)*{1,prob} built on
DVE and indirect-scattered into the pre-zeroed dense outputs.
"""
import numpy as np

import concourse.bacc as bacc
import concourse.mybir as mybir
from concourse.tile import TileContext
from concourse.bass_utils import run_bass_kernel_spmd

P = 128          # partitions
B, N, D, E, C = 4, 2048, 2048, 8, 512
CAP = 640
NCORES = 8
TOK = (B * N) // NCORES     # tokens per core = 1024
NT = TOK // P               # token tiles per core = 8
KT = D // P                 # contraction tiles = 16

WSCALE = 1024.0   # pre-scale W so its fp16 lo bits stay in normal range
THETA = 0.0128    # flag margin on fp16-pass logits (8x measured max abs err)
TIE_TOL = 1.2e-5  # below fp32-reference resolution: tie -> lower expert index

f32 = mybir.dt.float32

_cache = {}
LAST_IN_MAPS_A = None   # kept for test harness re-runs/profiling
LAST_IN_MAPS_B = None


def _build_phase_a():
    f16 = mybir.dt.float16
    nc = bacc.Bacc("TRN2", target_bir_lowering=False, debug=False, num_devices=NCORES)
    xT = nc.dram_tensor("xT", [D, TOK], f16, kind="ExternalInput")
    w = nc.dram_tensor("w", [E, D, C], f16, kind="ExternalInput")
    ss_out = nc.dram_tensor("ss", [TOK, E], f32, kind="ExternalOutput")

    with TileContext(nc) as tc:
        with (
            tc.tile_pool(name="const", bufs=1) as cpool,
            tc.tile_pool(name="wbuf", bufs=2) as wpool,
            tc.tile_pool(name="work", bufs=3) as spool,
            tc.tile_pool(name="psum", bufs=8, space="PSUM") as psum,
        ):
            # PE warmup: dummy matmuls on a memset tile keep the PE busy from
            # ~t=0 so the HAM clock-gate opens (K=8/8) during the initial DMA
            # wait and the first real matmuls run warm instead of idling
            # ~16us for the first chunks.
            NWARM = 24
            warm = cpool.tile([P, C], f16, tag="warm")
            nc.vector.memset(warm[:], 0.0)
            for _ in range(NWARM):
                wps = psum.tile([P, C], f32, space="PSUM", tag="ps")
                nc.tensor.matmul(wps[:], lhsT=warm[:, 0:P], rhs=warm[:],
                                 start=True, stop=True)

            # x^T resident in variable k-chunk tiles; W per expert likewise
            # (double-buffered). DMAs are issued in consumption order and the
            # first chunk is a single k-block, so the first matmuls wait on
            # ~0.4MB instead of the full 21MB. x goes on the scalar-engine
            # HWDGE queue so it streams in parallel with W on the sync queue.
            CHUNKS = [1, 3, 4, 4, 4]           # k-blocks per chunk, sums to KT
            CH0 = [sum(CHUNKS[:i]) for i in range(len(CHUNKS))]  # chunk k-starts
            NCH = len(CHUNKS)

            def _x_chunk(q):
                nk = CHUNKS[q]
                name = f"xq{q}"
                tile_ = cpool.tile([P, nk * TOK], f16, tag=name, name=name)
                nc.scalar.dma_start(
                    out=tile_[:].rearrange("p (k n) -> p k n", k=nk),
                    in_=xT.ap()[CH0[q] * P:(CH0[q] + nk) * P, :]
                        .rearrange("(k p) n -> p k n", p=P),
                )
                return tile_

            def _w_chunk(e, q):
                nk = CHUNKS[q]
                tile_ = wpool.tile([P, nk * C], f16, tag=f"wq{q}", name=f"w{e}_{q}")
                nc.sync.dma_start(
                    out=tile_[:].rearrange("p (k c) -> p k c", k=nk),
                    in_=w.ap()[e, CH0[q] * P:(CH0[q] + nk) * P, :]
                        .rearrange("(k p) c -> p k c", p=P),
                )
                return tile_

            # consumption-order issue: W(e0,q0), x(q0), W(e0,q1), x(q1), ...
            w0_q, x_q = [], []
            for q in range(NCH):
                w0_q.append(_w_chunk(0, q))
                x_q.append(_x_chunk(q))

            # per-token-tile sum-of-squares accumulators [128, E]
            ss_tiles = [cpool.tile([P, E], f32, tag=f"ss{t}", name=f"ss{t}")
                        for t in range(NT)]

            # ---- matmul phase: for each expert, 8 token tiles x 16 k-tiles ----
            for e in range(E):
                w_q = w0_q if e == 0 else [_w_chunk(e, q) for q in range(NCH)]
                for t in range(NT):
                    ps = psum.tile([P, C], f32, space="PSUM", tag="ps")
                    for k in range(KT):
                        q = max(i for i in range(NCH) if CH0[i] <= k)
                        kq = k - CH0[q]
                        nc.tensor.matmul(
                            ps[:],
                            lhsT=x_q[q][:, kq * TOK + t * P: kq * TOK + (t + 1) * P],
                            rhs=w_q[q][:, kq * C:(kq + 1) * C],
                            start=(k == 0),
                            stop=(k == KT - 1),
                        )
                    sq = spool.tile([P, C], f32, tag="sq")
                    nc.scalar.activation(sq[:], ps[:], mybir.ActivationFunctionType.Square)
                    red8 = spool.tile([P, 8], f32, tag="red8")
                    nc.vector.tensor_reduce(
                        red8[:], sq[:].rearrange("p (g c) -> p g c", g=8),
                        axis=mybir.AxisListType.X, op=mybir.AluOpType.add,
                    )
                    nc.vector.tensor_reduce(
                        ss_tiles[t][:, e:e + 1], red8[:],
                        axis=mybir.AxisListType.X, op=mybir.AluOpType.add,
                    )

            for t in range(NT):
                nc.sync.dma_start(out=ss_out.ap()[t * P:(t + 1) * P, :],
                                  in_=ss_tiles[t][:])
    nc.compile()
    return nc


def _build_phase_b(cap=CAP):
    """Scatter expansion: dispatch/combine have <=2 nonzero (t,e) rows per
    token; build only those 2048 rows each and indirect-scatter them into the
    pre-zeroed output. The dispatch and combine rows for one (token, choice)
    are fused into a single [2*cap] row of an interleaved output tensor, so
    one indirect DMA (one serial ~1.1us Q7 descriptor emission) covers both
    -- the host splits the halves during assembly."""
    import concourse.bass as bass
    i32 = mybir.dt.int32
    NR = 2 * TOK          # (token x choice) rows per core
    NG = NR // P          # 16 scatter groups of 128 rows
    nc = bacc.Bacc("TRN2", target_bir_lowering=False, debug=False, num_devices=NCORES)
    sp = nc.dram_tensor("sp", [2 * NR, 1], f32, kind="ExternalInput")   # slots | probs
    ridx = nc.dram_tensor("ridx", [NR, 1], i32, kind="ExternalInput")
    dc = nc.dram_tensor("dc", [TOK * E, 2 * cap], f32, kind="ExternalOutput")

    with TileContext(nc) as tc:
        with (
            tc.tile_pool(name="const", bufs=1) as cpool,
            tc.tile_pool(name="work", bufs=6) as spool,
        ):
            iota_sb = cpool.tile([P, cap], f32, tag="iota")
            nc.gpsimd.iota(iota_sb[:], pattern=[[1, cap]], base=0,
                           channel_multiplier=0,
                           allow_small_or_imprecise_dtypes=True)
            # batched scatter inputs: [128, g] group-major columns; slots in
            # cols [0,NG), probs in cols [NG,2*NG)
            spt = cpool.tile([P, 2 * NG], f32, tag="spt")
            nc.sync.dma_start(out=spt[:].rearrange("p (s g) -> p s g", s=2),
                              in_=sp.ap()[:, 0].rearrange("(s g p) -> p s g", p=P, s=2))
            ri = cpool.tile([P, NG], i32, tag="ri")
            nc.scalar.dma_start(out=ri[:], in_=ridx.ap()[:, 0].rearrange("(g p) -> p g", p=P))
            for g in range(NG):
                row = spool.tile([P, 2 * cap], f32, tag="row")
                nc.vector.tensor_scalar(row[:, 0:cap], iota_sb[:], spt[:, g:g + 1], None,
                                        op0=mybir.AluOpType.is_equal)
                nc.vector.tensor_scalar(row[:, cap:2 * cap], iota_sb[:], spt[:, g:g + 1],
                                        spt[:, NG + g:NG + g + 1],
                                        op0=mybir.AluOpType.is_equal,
                                        op1=mybir.AluOpType.mult)
                nc.gpsimd.indirect_dma_start(
                    out=dc.ap()[:, :],
                    out_offset=bass.IndirectOffsetOnAxis(ap=ri[:, g:g + 1], axis=0),
                    in_=row[:], in_offset=None)
    nc.compile()
    return nc


def _get(name, builder):
    if name not in _cache:
        _cache[name] = builder()
    return _cache[name]


def kernel(token_inputs, bottleneck_weights, expert_capacity):
    x = np.ascontiguousarray(np.asarray(token_inputs, dtype=np.float32)).reshape(B * N, D)
    w = np.ascontiguousarray(np.asarray(bottleneck_weights, dtype=np.float32))
    cap = int(expert_capacity)
    assert cap > 0

    w16 = (w * WSCALE).astype(np.float16)
    core_ids = list(range(NCORES))
    in_maps_a = []
    for c in core_ids:
        shard_t = np.ascontiguousarray(x[c * TOK:(c + 1) * TOK].T)   # [2048, 1024]
        in_maps_a.append({"xT": shard_t.astype(np.float16), "w": w16})

    global LAST_IN_MAPS_A, LAST_IN_MAPS_B
    LAST_IN_MAPS_A = in_maps_a
    nc_a = _get("a", _build_phase_a)
    res_a = run_bass_kernel_spmd(nc_a, in_maps_a, core_ids)

    # ---- host glue: logits -> decisions (with fp64 refinement of near-ties),
    # exact capacity cumsums, cross-core offsets, phase-B scatter tables.
    ss = np.concatenate([np.asarray(res_a.results[c]["ss"], np.float64)
                         for c in core_ids], axis=0) / (WSCALE * WSCALE)
    l = np.sqrt(ss)                                # [8192, E] fp16-pass logits

    order = np.argsort(-l, axis=1, kind="stable")
    l_srt = np.take_along_axis(l, order, 1)
    margin = np.minimum(l_srt[:, 0] - l_srt[:, 1], l_srt[:, 1] - l_srt[:, 2])
    flagged = np.where(margin < THETA)[0]

    if flagged.size:
        # exact logits for the ambiguous tokens (fp64 ~ exact at this scale)
        xf = x[flagged].astype(np.float64)
        wf = np.ascontiguousarray(w.astype(np.float64).transpose(1, 0, 2)).reshape(D, E * C)
        af = (xf @ wf).reshape(-1, E, C)
        l[flagged] = np.sqrt((af * af).sum(-1))
        # re-rank flagged tokens; near-exact ties (below the fp32 resolution of
        # the reference) go to the lower expert index, matching top_k on probs
        for i in flagged:
            li = l[i]
            o = np.argsort(-li, kind="stable")
            for _ in range(E):
                moved = False
                for j in range(E - 1):
                    if (li[o[j]] - li[o[j + 1]] < TIE_TOL) and o[j] > o[j + 1]:
                        o[j], o[j + 1] = o[j + 1], o[j]
                        moved = True
                if not moved:
                    break
            order[i] = o

    e0 = order[:, 0]
    e1 = order[:, 1]
    ex = np.exp(l - l.max(axis=1, keepdims=True))
    probs = ex / ex.sum(axis=1, keepdims=True)
    arN = np.arange(B * N)
    p0 = probs[arN, e0].astype(np.float32)
    p1 = probs[arN, e1].astype(np.float32)

    # exact integer priorities, replicating the reference's k-major cumsum
    slot0 = np.empty(B * N, np.int64)
    slot1 = np.empty(B * N, np.int64)
    arn = np.arange(N)
    for b in range(B):
        sl_ = slice(b * N, (b + 1) * N)
        idx = np.concatenate([e0[sl_], e1[sl_]])
        oh = (idx[:, None] == np.arange(E)[None, :]).astype(np.int64)
        pri = np.cumsum(oh, axis=0) * oh - 1
        slot0[sl_] = pri[:N][arn, e0[sl_]]
        slot1[sl_] = pri[N:][arn, e1[sl_]]

    ar = np.arange(TOK)
    in_maps_b = []
    for c in core_ids:
        sl_ = slice(c * TOK, (c + 1) * TOK)
        in_maps_b.append({
            "sp": np.concatenate([slot0[sl_], slot1[sl_], p0[sl_], p1[sl_]])
                    .astype(np.float32)[:, None],
            "ridx": np.concatenate([ar * E + e0[sl_], ar * E + e1[sl_]]).astype(np.int32)[:, None],
        })

    LAST_IN_MAPS_B = in_maps_b
    nc_b = _get(f"b{cap}", lambda: _build_phase_b(cap))
    res_b = run_bass_kernel_spmd(nc_b, in_maps_b, core_ids)

    out = np.empty((2, B, N, E, cap), np.float32)
    for c in core_ids:
        b, h = c // 2, c % 2
        sl_ = slice(h * TOK, (h + 1) * TOK)
        dc = res_b.results[c]["dc"].reshape(TOK, E, 2 * cap)
        out[0, b, sl_] = dc[:, :, :cap]
        out[1, b, sl_] = dc[:, :, cap:]
    return out


# revision 11
# speedup vs baseline: 1.1636x; 1.1636x over previous
"""MoE router (AutonomousRouter) for TRN2, 8 NeuronCores.

Computes reference:
    act    = einsum('bnd,edc->bnec', x, W)          B,N,D,E,C = 4,2048,2048,8,512
    logits = ||act||_2 over c                       [B,N,E]
    probs  = softmax(logits, -1)
    top-2 routing with capacity 640 (priority = order within k-major (choice, token) sequence)
    out    = stack([dispatch, combine])             [2,B,N,E,640] fp32

Sharding: data-parallel over tokens; core i <- tokens [i*1024, (i+1)*1024) of the
flattened [8192] token axis (= batch b=i//2, half i%2). Weights replicated.

Error budget: the graded metric is absmax/scale with scale ~1 (dispatch ones), so
routing DECISIONS (ordered top-2 per token) must match the fp32 reference exactly,
while combine probabilities only need ~2e-2 absolute. A single fp16 matmul pass
(11-bit mantissa, exact products accumulated in fp32 PSUM) gives logits to ~1.6e-3
absolute — enough to decide all tokens whose top-3 margin exceeds THETA, and to get
probs to ~3e-4. The few hundred tokens with margin < THETA are re-decided on host in
fp64 during the inter-phase glue (the same place the cross-core cumsum offsets are
combined), with near-exact ties (< TIE_TOL, i.e. below fp32 reference resolution)
broken toward the lower expert index. This replaces the previous 3x bf16-split
matmul (hh+hl+lh) with a single fp16 pass -- 3x less PE work.

Phase A (device): fp16 matmul -> square -> reduce = sum-of-squares logits proxy.
Host glue: logits/softmax/top-2 + fp64 refinement of near-tie tokens + exact
integer capacity cumsums + cross-core offsets.
Phase B (device): per-(token,choice) one-hot rows (iota==slot)*{1,prob} built on
DVE and indirect-scattered into the pre-zeroed dense outputs.
"""
import numpy as np

import concourse.bacc as bacc
import concourse.mybir as mybir
from concourse.tile import TileContext
from concourse.bass_utils import run_bass_kernel_spmd

P = 128          # partitions
B, N, D, E, C = 4, 2048, 2048, 8, 512
CAP = 640
NCORES = 8
TOK = (B * N) // NCORES     # tokens per core = 1024
NT = TOK // P               # token tiles per core = 8
KT = D // P                 # contraction tiles = 16

WSCALE = 1024.0   # pre-scale W so its fp16 lo bits stay in normal range
THETA = 0.0128    # flag margin on fp16-pass logits (8x measured max abs err)
TIE_TOL = 1.2e-5  # below fp32-reference resolution: tie -> lower expert index

f32 = mybir.dt.float32

_cache = {}
LAST_IN_MAPS_A = None   # kept for test harness re-runs/profiling
LAST_IN_MAPS_B = None


def _build_phase_a():
    f16 = mybir.dt.float16
    nc = bacc.Bacc("TRN2", target_bir_lowering=False, debug=False, num_devices=NCORES)
    xT = nc.dram_tensor("xT", [D, TOK], f16, kind="ExternalInput")
    w = nc.dram_tensor("w", [E, D, C], f16, kind="ExternalInput")
    ss_out = nc.dram_tensor("ss", [TOK, E], f32, kind="ExternalOutput")

    with TileContext(nc) as tc:
        with (
            tc.tile_pool(name="const", bufs=1) as cpool,
            tc.tile_pool(name="wbuf", bufs=2) as wpool,
            tc.tile_pool(name="work", bufs=3) as spool,
            tc.tile_pool(name="psum", bufs=8, space="PSUM") as psum,
        ):
            # PE warmup: dummy matmuls on a memset tile keep the PE busy while
            # the first chunks stream in, so the HAM clock-gate opens (K=8/8)
            # before the real matmuls start and they never run at 1.2GHz.
            NWARM = 12
            warm = cpool.tile([P, C], f16, tag="warm")
            nc.vector.memset(warm[:], 0.0)
            for _ in range(NWARM):
                wps = psum.tile([P, C], f32, space="PSUM", tag="ps")
                nc.tensor.matmul(wps[:], lhsT=warm[:, 0:P], rhs=warm[:],
                                 start=True, stop=True)

            # x^T resident in variable k-chunk tiles; W per expert likewise
            # (double-buffered). All DMAs go on one HWDGE queue in exact
            # consumption order (w0q0, xq0, w0q1, xq1, ..., w1, w2, ...), so
            # with the k-outer matmul loop below the PE consumes chunk q
            # slower than the stream delivers chunk q+1 and never stalls.
            CHUNKS = [1, 3, 4, 4, 4]           # k-blocks per chunk, sums to KT
            CH0 = [sum(CHUNKS[:i]) for i in range(len(CHUNKS))]  # chunk k-starts
            NCH = len(CHUNKS)

            def _x_chunk(q):
                nk = CHUNKS[q]
                name = f"xq{q}"
                tile_ = cpool.tile([P, nk * TOK], f16, tag=name, name=name)
                nc.sync.dma_start(
                    out=tile_[:].rearrange("p (k n) -> p k n", k=nk),
                    in_=xT.ap()[CH0[q] * P:(CH0[q] + nk) * P, :]
                        .rearrange("(k p) n -> p k n", p=P),
                )
                return tile_

            def _w_chunk(e, q):
                nk = CHUNKS[q]
                tile_ = wpool.tile([P, nk * C], f16, tag=f"wq{q}", name=f"w{e}_{q}")
                nc.sync.dma_start(
                    out=tile_[:].rearrange("p (k c) -> p k c", k=nk),
                    in_=w.ap()[e, CH0[q] * P:(CH0[q] + nk) * P, :]
                        .rearrange("(k p) c -> p k c", p=P),
                )
                return tile_

            # consumption-order issue: W(e0,q0), x(q0), W(e0,q1), x(q1), ...
            w0_q, x_q = [], []
            for q in range(NCH):
                w0_q.append(_w_chunk(0, q))
                x_q.append(_x_chunk(q))

            # per-token-tile sum-of-squares accumulators [128, E]
            ss_tiles = [cpool.tile([P, E], f32, tag=f"ss{t}", name=f"ss{t}")
                        for t in range(NT)]

            # ---- matmul phase: per expert, k-outer over all 8 token-tile PSUM
            # banks, so k-chunk q is only needed after (q-1) chunks of PE work
            # and the single in-order DMA stream always runs ahead of the PE.
            for e in range(E):
                w_q = w0_q if e == 0 else [_w_chunk(e, q) for q in range(NCH)]
                ps_t = [psum.tile([P, C], f32, space="PSUM", tag="ps",
                                  name=f"ps{e}_{t}")
                        for t in range(NT)]
                for k in range(KT):
                    q = max(i for i in range(NCH) if CH0[i] <= k)
                    kq = k - CH0[q]
                    for t in range(NT):
                        nc.tensor.matmul(
                            ps_t[t][:],
                            lhsT=x_q[q][:, kq * TOK + t * P: kq * TOK + (t + 1) * P],
                            rhs=w_q[q][:, kq * C:(kq + 1) * C],
                            start=(k == 0),
                            stop=(k == KT - 1),
                        )
                for t in range(NT):
                    sq = spool.tile([P, C], f32, tag="sq")
                    nc.scalar.activation(sq[:], ps_t[t][:], mybir.ActivationFunctionType.Square)
                    red8 = spool.tile([P, 8], f32, tag="red8")
                    nc.vector.tensor_reduce(
                        red8[:], sq[:].rearrange("p (g c) -> p g c", g=8),
                        axis=mybir.AxisListType.X, op=mybir.AluOpType.add,
                    )
                    nc.vector.tensor_reduce(
                        ss_tiles[t][:, e:e + 1], red8[:],
                        axis=mybir.AxisListType.X, op=mybir.AluOpType.add,
                    )

            for t in range(NT):
                nc.sync.dma_start(out=ss_out.ap()[t * P:(t + 1) * P, :],
                                  in_=ss_tiles[t][:])
    nc.compile()
    return nc


def _build_phase_b(cap=CAP):
    """Scatter expansion, element-pair form. The dense [2,TOK,E,cap] output
    per core has <=2 nonzero (dispatch, combine) element pairs per token, at
    rows the host fully precomputes: row = (token*E + e_k)*cap + slot_k of a
    [TOK*E*cap, 2] view whose last axis is (dispatch, combine). The device
    just indirect-scatters 8-byte (1.0, prob) pairs -- 16 serial ~1.1us Q7
    descriptor emissions and ~16KB of writes, instead of building/writing
    10.5MB of one-hot rows (scattered-row HBM writes measured only
    ~130GB/s, which made the row variant ~80us+). Capacity-dropped tokens
    get an out-of-range row index and are skipped via bounds_check."""
    import concourse.bass as bass
    i32 = mybir.dt.int32
    NR = 2 * TOK          # (token x choice) scatter elements per core
    NG = NR // P          # 16 scatter groups of 128
    ROWS = TOK * E * cap
    nc = bacc.Bacc("TRN2", target_bir_lowering=False, debug=False, num_devices=NCORES)
    vals = nc.dram_tensor("vals", [NR, 2], f32, kind="ExternalInput")   # (1.0, prob)
    ro = nc.dram_tensor("ro", [NR, 1], i32, kind="ExternalInput")       # target rows
    dc = nc.dram_tensor("dc", [ROWS, 2], f32, kind="ExternalOutput")

    with TileContext(nc) as tc:
        with tc.tile_pool(name="const", bufs=1) as cpool:
            val = cpool.tile([P, NG * 2], f32, tag="val")
            nc.sync.dma_start(out=val[:].rearrange("p (g t) -> p g t", t=2),
                              in_=vals.ap().rearrange("(g p) t -> p g t", p=P))
            rot = cpool.tile([P, NG], i32, tag="rot")
            nc.scalar.dma_start(out=rot[:], in_=ro.ap()[:, 0].rearrange("(g p) -> p g", p=P))
            for g in range(NG):
                nc.gpsimd.indirect_dma_start(
                    out=dc.ap()[:, :],
                    out_offset=bass.IndirectOffsetOnAxis(ap=rot[:, g:g + 1], axis=0),
                    in_=val[:, 2 * g:2 * g + 2], in_offset=None,
                    bounds_check=ROWS - 1, oob_is_err=False)
    nc.compile()
    return nc


def _get(name, builder):
    if name not in _cache:
        _cache[name] = builder()
    return _cache[name]


def kernel(token_inputs, bottleneck_weights, expert_capacity):
    x = np.ascontiguousarray(np.asarray(token_inputs, dtype=np.float32)).reshape(B * N, D)
    w = np.ascontiguousarray(np.asarray(bottleneck_weights, dtype=np.float32))
    cap = int(expert_capacity)
    assert cap > 0

    w16 = (w * WSCALE).astype(np.float16)
    core_ids = list(range(NCORES))
    in_maps_a = []
    for c in core_ids:
        shard_t = np.ascontiguousarray(x[c * TOK:(c + 1) * TOK].T)   # [2048, 1024]
        in_maps_a.append({"xT": shard_t.astype(np.float16), "w": w16})

    global LAST_IN_MAPS_A, LAST_IN_MAPS_B
    LAST_IN_MAPS_A = in_maps_a
    nc_a = _get("a", _build_phase_a)
    res_a = run_bass_kernel_spmd(nc_a, in_maps_a, core_ids)

    # ---- host glue: logits -> decisions (with fp64 refinement of near-ties),
    # exact capacity cumsums, cross-core offsets, phase-B scatter tables.
    ss = np.concatenate([np.asarray(res_a.results[c]["ss"], np.float64)
                         for c in core_ids], axis=0) / (WSCALE * WSCALE)
    l = np.sqrt(ss)                                # [8192, E] fp16-pass logits

    order = np.argsort(-l, axis=1, kind="stable")
    l_srt = np.take_along_axis(l, order, 1)
    margin = np.minimum(l_srt[:, 0] - l_srt[:, 1], l_srt[:, 1] - l_srt[:, 2])
    flagged = np.where(margin < THETA)[0]

    if flagged.size:
        # exact logits for the ambiguous tokens (fp64 ~ exact at this scale)
        xf = x[flagged].astype(np.float64)
        wf = np.ascontiguousarray(w.astype(np.float64).transpose(1, 0, 2)).reshape(D, E * C)
        af = (xf @ wf).reshape(-1, E, C)
        l[flagged] = np.sqrt((af * af).sum(-1))
        # re-rank flagged tokens; near-exact ties (below the fp32 resolution of
        # the reference) go to the lower expert index, matching top_k on probs
        for i in flagged:
            li = l[i]
            o = np.argsort(-li, kind="stable")
            for _ in range(E):
                moved = False
                for j in range(E - 1):
                    if (li[o[j]] - li[o[j + 1]] < TIE_TOL) and o[j] > o[j + 1]:
                        o[j], o[j + 1] = o[j + 1], o[j]
                        moved = True
                if not moved:
                    break
            order[i] = o

    e0 = order[:, 0]
    e1 = order[:, 1]
    ex = np.exp(l - l.max(axis=1, keepdims=True))
    probs = ex / ex.sum(axis=1, keepdims=True)
    arN = np.arange(B * N)
    p0 = probs[arN, e0].astype(np.float32)
    p1 = probs[arN, e1].astype(np.float32)

    # exact integer priorities, replicating the reference's k-major cumsum
    slot0 = np.empty(B * N, np.int64)
    slot1 = np.empty(B * N, np.int64)
    arn = np.arange(N)
    for b in range(B):
        sl_ = slice(b * N, (b + 1) * N)
        idx = np.concatenate([e0[sl_], e1[sl_]])
        oh = (idx[:, None] == np.arange(E)[None, :]).astype(np.int64)
        pri = np.cumsum(oh, axis=0) * oh - 1
        slot0[sl_] = pri[:N][arn, e0[sl_]]
        slot1[sl_] = pri[N:][arn, e1[sl_]]

    ar = np.arange(TOK)
    OOB = np.int64(1) << 30
    in_maps_b = []
    for c in core_ids:
        sl_ = slice(c * TOK, (c + 1) * TOK)
        slots = np.concatenate([slot0[sl_], slot1[sl_]])
        ridx = np.concatenate([ar * E + e0[sl_], ar * E + e1[sl_]])
        rows = np.where(slots < cap, ridx * cap + slots, OOB).astype(np.int32)
        vals = np.empty((2 * TOK, 2), np.float32)
        vals[:, 0] = 1.0
        vals[:, 1] = np.concatenate([p0[sl_], p1[sl_]])
        in_maps_b.append({"vals": vals, "ro": rows[:, None]})

    LAST_IN_MAPS_B = in_maps_b
    nc_b = _get(f"b{cap}", lambda: _build_phase_b(cap))
    res_b = run_bass_kernel_spmd(nc_b, in_maps_b, core_ids)

    out = np.empty((2, B, N, E, cap), np.float32)
    for c in core_ids:
        b, h = c // 2, c % 2
        sl_ = slice(h * TOK, (h + 1) * TOK)
        dcr = res_b.results[c]["dc"].reshape(TOK, E, cap, 2)
        out[0, b, sl_] = dcr[..., 0]
        out[1, b, sl_] = dcr[..., 1]
    return out


# revision 17
# speedup vs baseline: 1.2168x; 1.0457x over previous
"""MoE router (AutonomousRouter) for TRN2, 8 NeuronCores.

Computes reference:
    act    = einsum('bnd,edc->bnec', x, W)          B,N,D,E,C = 4,2048,2048,8,512
    logits = ||act||_2 over c                       [B,N,E]
    probs  = softmax(logits, -1)
    top-2 routing with capacity 640 (priority = order within k-major (choice, token) sequence)
    out    = stack([dispatch, combine])             [2,B,N,E,640] fp32

Sharding: data-parallel over tokens; core i <- tokens [i*1024, (i+1)*1024) of the
flattened [8192] token axis (= batch b=i//2, half i%2). Weights replicated.

Error budget: the graded metric is absmax/scale with scale ~1 (dispatch ones), so
routing DECISIONS (ordered top-2 per token) must match the fp32 reference exactly,
while combine probabilities only need ~2e-2 absolute. A single fp16 matmul pass
(11-bit mantissa, exact products accumulated in fp32 PSUM) gives logits to ~1.6e-3
absolute — enough to decide all tokens whose top-3 margin exceeds THETA, and to get
probs to ~3e-4. The few hundred tokens with margin < THETA are re-decided on host in
fp64 during the inter-phase glue (the same place the cross-core cumsum offsets are
combined), with near-exact ties (< TIE_TOL, i.e. below fp32 reference resolution)
broken toward the lower expert index. This replaces the previous 3x bf16-split
matmul (hh+hl+lh) with a single fp16 pass -- 3x less PE work.

Phase A (device): fp16 matmul -> square -> reduce = sum-of-squares logits proxy.
Host glue: logits/softmax/top-2 + fp64 refinement of near-tie tokens + exact
integer capacity cumsums + cross-core offsets.
Phase B (device): per-(token,choice) one-hot rows (iota==slot)*{1,prob} built on
DVE and indirect-scattered into the pre-zeroed dense outputs.
"""
import numpy as np

import concourse.bacc as bacc
import concourse.mybir as mybir
from concourse.tile import TileContext
from concourse.bass_utils import run_bass_kernel_spmd

P = 128          # partitions
B, N, D, E, C = 4, 2048, 2048, 8, 512
CAP = 640
NCORES = 8
TOK = (B * N) // NCORES     # tokens per core = 1024
NT = TOK // P               # token tiles per core = 8
KT = D // P                 # contraction tiles = 16

WSCALE = 1024.0   # pre-scale W so its fp16 lo bits stay in normal range
THETA = 0.0128    # flag margin on fp16-pass logits (8x measured max abs err)
TIE_TOL = 1.2e-5  # below fp32-reference resolution: tie -> lower expert index

f32 = mybir.dt.float32

_cache = {}
LAST_IN_MAPS_A = None   # kept for test harness re-runs/profiling
LAST_IN_MAPS_B = None


def _build_phase_a():
    f16 = mybir.dt.float16
    nc = bacc.Bacc("TRN2", target_bir_lowering=False, debug=False, num_devices=NCORES)
    xT = nc.dram_tensor("xT", [D, TOK], f16, kind="ExternalInput")
    w = nc.dram_tensor("w", [E, D, C], f16, kind="ExternalInput")
    ss_out = nc.dram_tensor("ss", [E, TOK], f32, kind="ExternalOutput")

    with TileContext(nc) as tc:
        with (
            tc.tile_pool(name="const", bufs=1) as cpool,
            tc.tile_pool(name="wbuf", bufs=2) as wpool,
            tc.tile_pool(name="work", bufs=3) as spool,
            tc.tile_pool(name="psum", bufs=8, space="PSUM") as psum,
        ):
            # PE warmup: dummy matmuls on a memset tile keep the PE busy while
            # the first chunks stream in, so the HAM clock-gate opens (K=8/8)
            # before the real matmuls start and they never run at 1.2GHz.
            NWARM = 12
            warm = cpool.tile([P, C], f16, tag="warm")
            nc.vector.memset(warm[:], 0.0)
            for _ in range(NWARM):
                wps = psum.tile([P, C], f32, space="PSUM", tag="ps")
                nc.tensor.matmul(wps[:], lhsT=warm[:, 0:P], rhs=warm[:],
                                 start=True, stop=True)

            # x^T resident in variable k-chunk tiles; W per expert likewise
            # (double-buffered). All DMAs go on one HWDGE queue in exact
            # consumption order (w0q0, xq0, w0q1, xq1, ..., w1, w2, ...), so
            # with the k-outer matmul loop below the PE consumes chunk q
            # slower than the stream delivers chunk q+1 and never stalls.
            CHUNKS = [1, 2, 3, 4, 3, 3]        # k-blocks per chunk, sums to KT
            CH0 = [sum(CHUNKS[:i]) for i in range(len(CHUNKS))]  # chunk k-starts
            NCH = len(CHUNKS)

            def _x_chunk(q):
                nk = CHUNKS[q]
                name = f"xq{q}"
                tile_ = cpool.tile([P, nk * TOK], f16, tag=name, name=name)
                nc.sync.dma_start(
                    out=tile_[:].rearrange("p (k n) -> p k n", k=nk),
                    in_=xT.ap()[CH0[q] * P:(CH0[q] + nk) * P, :]
                        .rearrange("(k p) n -> p k n", p=P),
                )
                return tile_

            def _w_chunk(e, q):
                nk = CHUNKS[q]
                tile_ = wpool.tile([P, nk * C], f16, tag=f"wq{q}", name=f"w{e}_{q}")
                nc.sync.dma_start(
                    out=tile_[:].rearrange("p (k c) -> p k c", k=nk),
                    in_=w.ap()[e, CH0[q] * P:(CH0[q] + nk) * P, :]
                        .rearrange("(k p) c -> p k c", p=P),
                )
                return tile_

            # consumption-order issue: W(e0,q0), x(q0), W(e0,q1), x(q1), ...
            w0_q, x_q = [], []
            for q in range(NCH):
                w0_q.append(_w_chunk(0, q))
                x_q.append(_x_chunk(q))

            def _drain(ps, sse, t):
                sq = spool.tile([P, C], f32, tag="sq", name="sq")
                nc.scalar.activation(sq[:], ps[:], mybir.ActivationFunctionType.Square)
                red8 = spool.tile([P, 8], f32, tag="red8", name="red8")
                nc.vector.tensor_reduce(
                    red8[:], sq[:].rearrange("p (g c) -> p g c", g=8),
                    axis=mybir.AxisListType.X, op=mybir.AluOpType.add,
                )
                nc.vector.tensor_reduce(
                    sse[:, t:t + 1], red8[:],
                    axis=mybir.AxisListType.X, op=mybir.AluOpType.add,
                )

            # ---- matmul phase: per expert, k-outer over all 8 token-tile PSUM
            # banks, so k-chunk q is only needed after (q-1) chunks of PE work
            # and the single in-order DMA stream always runs ahead of the PE.
            # The last expert runs t-outer instead (its data is resident by
            # then), so the PSUM drain overlaps the matmuls and the kernel
            # doesn't end with 8 serial square+reduce passes. Each expert's
            # sum-of-squares row DMAs out as soon as that expert finishes.
            for e in range(E):
                w_q = w0_q if e == 0 else [_w_chunk(e, q) for q in range(NCH)]
                sse = cpool.tile([P, NT], f32, tag=f"ssE{e}", name=f"ssE{e}")
                if e < E - 1:
                    ps_t = [psum.tile([P, C], f32, space="PSUM", tag="ps",
                                      name=f"ps{e}_{t}")
                            for t in range(NT)]
                    for k in range(KT):
                        q = max(i for i in range(NCH) if CH0[i] <= k)
                        kq = k - CH0[q]
                        for t in range(NT):
                            nc.tensor.matmul(
                                ps_t[t][:],
                                lhsT=x_q[q][:, kq * TOK + t * P: kq * TOK + (t + 1) * P],
                                rhs=w_q[q][:, kq * C:(kq + 1) * C],
                                start=(k == 0),
                                stop=(k == KT - 1),
                            )
                    for t in range(NT):
                        _drain(ps_t[t], sse, t)
                else:
                    for t in range(NT):
                        ps = psum.tile([P, C], f32, space="PSUM", tag="ps",
                                       name=f"ps{e}_{t}")
                        for k in range(KT):
                            q = max(i for i in range(NCH) if CH0[i] <= k)
                            kq = k - CH0[q]
                            nc.tensor.matmul(
                                ps[:],
                                lhsT=x_q[q][:, kq * TOK + t * P: kq * TOK + (t + 1) * P],
                                rhs=w_q[q][:, kq * C:(kq + 1) * C],
                                start=(k == 0),
                                stop=(k == KT - 1),
                            )
                        _drain(ps, sse, t)
                nc.sync.dma_start(
                    out=ss_out.ap()[e, :].rearrange("(t p) -> p t", p=P),
                    in_=sse[:])
    nc.compile()
    return nc


def _build_phase_b(cap=CAP):
    """Scatter expansion, element-pair form. The dense [2,TOK,E,cap] output
    per core has <=2 nonzero (dispatch, combine) element pairs per token, at
    rows the host fully precomputes: row = (token*E + e_k)*cap + slot_k of a
    [TOK*E*cap, 2] view whose last axis is (dispatch, combine). The device
    just indirect-scatters 8-byte (1.0, prob) pairs -- 16 serial ~1.1us Q7
    descriptor emissions and ~16KB of writes, instead of building/writing
    10.5MB of one-hot rows (scattered-row HBM writes measured only
    ~130GB/s, which made the row variant ~80us+). Capacity-dropped tokens
    get an out-of-range row index and are skipped via bounds_check."""
    import concourse.bass as bass
    i32 = mybir.dt.int32
    NR = 2 * TOK          # (token x choice) scatter elements per core
    NG = NR // P          # 16 scatter groups of 128
    NSPLIT = 4            # independent output tensors to break the WAW chain
    ROWS = TOK * E * cap
    nc = bacc.Bacc("TRN2", target_bir_lowering=False, debug=False, num_devices=NCORES)
    vals = nc.dram_tensor("vals", [NR, 2], f32, kind="ExternalInput")   # (1.0, prob)
    ro = nc.dram_tensor("ro", [NR, 1], i32, kind="ExternalInput")       # target rows
    # Tile serializes same-tensor indirect writes on the completion
    # semaphore (~2.5us per call); round-robining the groups over NSPLIT
    # disjoint tensors (host sums them) leaves only the ~1.1us Q7
    # descriptor emission serial.
    dcs = [nc.dram_tensor(f"dc{j}", [ROWS, 2], f32, kind="ExternalOutput")
           for j in range(NSPLIT)]

    with TileContext(nc) as tc:
        with tc.tile_pool(name="const", bufs=1) as cpool:
            # dummy all-OOB scatter issued before the input DMAs land: warms
            # the SWDGE/indirect path off the critical path
            dum = cpool.tile([P, 2], i32, tag="dum")
            nc.gpsimd.iota(dum[:], pattern=[[1, 2]], base=(1 << 30),
                           channel_multiplier=0)
            nc.gpsimd.indirect_dma_start(
                out=dcs[0].ap()[:, :],
                out_offset=bass.IndirectOffsetOnAxis(ap=dum[:, 0:1], axis=0),
                in_=dum[:, :], in_offset=None,
                bounds_check=ROWS - 1, oob_is_err=False)
            val = cpool.tile([P, NG * 2], f32, tag="val")
            nc.sync.dma_start(out=val[:].rearrange("p (g t) -> p g t", t=2),
                              in_=vals.ap().rearrange("(g p) t -> p g t", p=P))
            rot = cpool.tile([P, NG], i32, tag="rot")
            nc.scalar.dma_start(out=rot[:], in_=ro.ap()[:, 0].rearrange("(g p) -> p g", p=P))
            for g in range(NG):
                nc.gpsimd.indirect_dma_start(
                    out=dcs[g % NSPLIT].ap()[:, :],
                    out_offset=bass.IndirectOffsetOnAxis(ap=rot[:, g:g + 1], axis=0),
                    in_=val[:, 2 * g:2 * g + 2], in_offset=None,
                    bounds_check=ROWS - 1, oob_is_err=False)
    nc.compile()
    return nc


def _get(name, builder):
    if name not in _cache:
        _cache[name] = builder()
    return _cache[name]


def kernel(token_inputs, bottleneck_weights, expert_capacity):
    x = np.ascontiguousarray(np.asarray(token_inputs, dtype=np.float32)).reshape(B * N, D)
    w = np.ascontiguousarray(np.asarray(bottleneck_weights, dtype=np.float32))
    cap = int(expert_capacity)
    assert cap > 0

    w16 = (w * WSCALE).astype(np.float16)
    core_ids = list(range(NCORES))
    in_maps_a = []
    for c in core_ids:
        shard_t = np.ascontiguousarray(x[c * TOK:(c + 1) * TOK].T)   # [2048, 1024]
        in_maps_a.append({"xT": shard_t.astype(np.float16), "w": w16})

    global LAST_IN_MAPS_A, LAST_IN_MAPS_B
    LAST_IN_MAPS_A = in_maps_a
    nc_a = _get("a", _build_phase_a)
    res_a = run_bass_kernel_spmd(nc_a, in_maps_a, core_ids)

    # ---- host glue: logits -> decisions (with fp64 refinement of near-ties),
    # exact capacity cumsums, cross-core offsets, phase-B scatter tables.
    ss = np.concatenate([np.asarray(res_a.results[c]["ss"], np.float64).T
                         for c in core_ids], axis=0) / (WSCALE * WSCALE)
    l = np.sqrt(ss)                                # [8192, E] fp16-pass logits

    order = np.argsort(-l, axis=1, kind="stable")
    l_srt = np.take_along_axis(l, order, 1)
    margin = np.minimum(l_srt[:, 0] - l_srt[:, 1], l_srt[:, 1] - l_srt[:, 2])
    flagged = np.where(margin < THETA)[0]

    if flagged.size:
        # exact logits for the ambiguous tokens (fp64 ~ exact at this scale)
        xf = x[flagged].astype(np.float64)
        wf = np.ascontiguousarray(w.astype(np.float64).transpose(1, 0, 2)).reshape(D, E * C)
        af = (xf @ wf).reshape(-1, E, C)
        l[flagged] = np.sqrt((af * af).sum(-1))
        # re-rank flagged tokens; near-exact ties (below the fp32 resolution of
        # the reference) go to the lower expert index, matching top_k on probs
        for i in flagged:
            li = l[i]
            o = np.argsort(-li, kind="stable")
            for _ in range(E):
                moved = False
                for j in range(E - 1):
                    if (li[o[j]] - li[o[j + 1]] < TIE_TOL) and o[j] > o[j + 1]:
                        o[j], o[j + 1] = o[j + 1], o[j]
                        moved = True
                if not moved:
                    break
            order[i] = o

    e0 = order[:, 0]
    e1 = order[:, 1]
    ex = np.exp(l - l.max(axis=1, keepdims=True))
    probs = ex / ex.sum(axis=1, keepdims=True)
    arN = np.arange(B * N)
    p0 = probs[arN, e0].astype(np.float32)
    p1 = probs[arN, e1].astype(np.float32)

    # exact integer priorities, replicating the reference's k-major cumsum
    slot0 = np.empty(B * N, np.int64)
    slot1 = np.empty(B * N, np.int64)
    arn = np.arange(N)
    for b in range(B):
        sl_ = slice(b * N, (b + 1) * N)
        idx = np.concatenate([e0[sl_], e1[sl_]])
        oh = (idx[:, None] == np.arange(E)[None, :]).astype(np.int64)
        pri = np.cumsum(oh, axis=0) * oh - 1
        slot0[sl_] = pri[:N][arn, e0[sl_]]
        slot1[sl_] = pri[N:][arn, e1[sl_]]

    ar = np.arange(TOK)
    OOB = np.int64(1) << 30
    in_maps_b = []
    for c in core_ids:
        sl_ = slice(c * TOK, (c + 1) * TOK)
        slots = np.concatenate([slot0[sl_], slot1[sl_]])
        ridx = np.concatenate([ar * E + e0[sl_], ar * E + e1[sl_]])
        rows = np.where(slots < cap, ridx * cap + slots, OOB).astype(np.int32)
        vals = np.empty((2 * TOK, 2), np.float32)
        vals[:, 0] = 1.0
        vals[:, 1] = np.concatenate([p0[sl_], p1[sl_]])
        in_maps_b.append({"vals": vals, "ro": rows[:, None]})

    LAST_IN_MAPS_B = in_maps_b
    nc_b = _get(f"b{cap}", lambda: _build_phase_b(cap))
    res_b = run_bass_kernel_spmd(nc_b, in_maps_b, core_ids)

    out = np.empty((2, B, N, E, cap), np.float32)
    for c in core_ids:
        b, h = c // 2, c % 2
        sl_ = slice(h * TOK, (h + 1) * TOK)
        r = res_b.results[c]
        dcr = (r["dc0"] + r["dc1"] + r["dc2"] + r["dc3"]).reshape(TOK, E, cap, 2)
        out[0, b, sl_] = dcr[..., 0]
        out[1, b, sl_] = dcr[..., 1]
    return out


# revision 19
# speedup vs baseline: 1.2540x; 1.0306x over previous
"""MoE router (AutonomousRouter) for TRN2, 8 NeuronCores.

Computes reference:
    act    = einsum('bnd,edc->bnec', x, W)          B,N,D,E,C = 4,2048,2048,8,512
    logits = ||act||_2 over c                       [B,N,E]
    probs  = softmax(logits, -1)
    top-2 routing with capacity 640 (priority = order within k-major (choice, token) sequence)
    out    = stack([dispatch, combine])             [2,B,N,E,640] fp32

Sharding: data-parallel over tokens; core i <- tokens [i*1024, (i+1)*1024) of the
flattened [8192] token axis (= batch b=i//2, half i%2). Weights replicated.

Error budget: the graded metric is absmax/scale with scale ~1 (dispatch ones), so
routing DECISIONS (ordered top-2 per token) must match the fp32 reference exactly,
while combine probabilities only need ~2e-2 absolute. A single fp16 matmul pass
(11-bit mantissa, exact products accumulated in fp32 PSUM) gives logits to ~1.6e-3
absolute — enough to decide all tokens whose top-3 margin exceeds THETA, and to get
probs to ~3e-4. The few hundred tokens with margin < THETA are re-decided on host in
fp64 during the inter-phase glue (the same place the cross-core cumsum offsets are
combined), with near-exact ties (< TIE_TOL, i.e. below fp32 reference resolution)
broken toward the lower expert index. This replaces the previous 3x bf16-split
matmul (hh+hl+lh) with a single fp16 pass -- 3x less PE work.

Phase A (device): fp16 matmul -> square -> reduce = sum-of-squares logits proxy.
Host glue: logits/softmax/top-2 + fp64 refinement of near-tie tokens + exact
integer capacity cumsums + cross-core offsets.
Phase B (device): per-(token,choice) one-hot rows (iota==slot)*{1,prob} built on
DVE and indirect-scattered into the pre-zeroed dense outputs.
"""
import numpy as np

import concourse.bacc as bacc
import concourse.mybir as mybir
from concourse.tile import TileContext
from concourse.bass_utils import run_bass_kernel_spmd

P = 128          # partitions
B, N, D, E, C = 4, 2048, 2048, 8, 512
CAP = 640
NCORES = 8
TOK = (B * N) // NCORES     # tokens per core = 1024
NT = TOK // P               # token tiles per core = 8
KT = D // P                 # contraction tiles = 16

WSCALE = 1024.0   # pre-scale W so its fp16 lo bits stay in normal range
THETA = 0.0128    # flag margin on fp16-pass logits (8x measured max abs err)
TIE_TOL = 1.2e-5  # below fp32-reference resolution: tie -> lower expert index

f32 = mybir.dt.float32

_cache = {}
LAST_IN_MAPS_A = None   # kept for test harness re-runs/profiling
LAST_IN_MAPS_B = None


def _build_phase_a():
    f16 = mybir.dt.float16
    nc = bacc.Bacc("TRN2", target_bir_lowering=False, debug=False, num_devices=NCORES)
    xT = nc.dram_tensor("xT", [D, TOK], f16, kind="ExternalInput")
    w = nc.dram_tensor("w", [E, D, C], f16, kind="ExternalInput")
    ss_out = nc.dram_tensor("ss", [E, TOK], f32, kind="ExternalOutput")

    with TileContext(nc) as tc:
        with (
            tc.tile_pool(name="const", bufs=1) as cpool,
            tc.tile_pool(name="wbuf", bufs=2) as wpool,
            tc.tile_pool(name="work", bufs=3) as spool,
            tc.tile_pool(name="psum", bufs=8, space="PSUM") as psum,
        ):
            # PE warmup: dummy matmuls on a memset tile keep the PE busy while
            # the first chunks stream in, so the HAM clock-gate opens (K=8/8)
            # before the real matmuls start and they never run at 1.2GHz.
            NWARM = 12
            warm = cpool.tile([P, C], f16, tag="warm")
            nc.vector.memset(warm[:], 0.0)
            for _ in range(NWARM):
                wps = psum.tile([P, C], f32, space="PSUM", tag="ps")
                nc.tensor.matmul(wps[:], lhsT=warm[:, 0:P], rhs=warm[:],
                                 start=True, stop=True)

            # x^T resident in variable k-chunk tiles; W per expert likewise
            # (double-buffered). All DMAs go on one HWDGE queue in exact
            # consumption order (w0q0, xq0, w0q1, xq1, ..., w1, w2, ...), so
            # with the k-outer matmul loop below the PE consumes chunk q
            # slower than the stream delivers chunk q+1 and never stalls.
            CHUNKS = [1, 2, 3, 4, 3, 3]        # k-blocks per chunk, sums to KT
            CH0 = [sum(CHUNKS[:i]) for i in range(len(CHUNKS))]  # chunk k-starts
            NCH = len(CHUNKS)

            def _x_chunk(q):
                nk = CHUNKS[q]
                name = f"xq{q}"
                tile_ = cpool.tile([P, nk * TOK], f16, tag=name, name=name)
                nc.sync.dma_start(
                    out=tile_[:].rearrange("p (k n) -> p k n", k=nk),
                    in_=xT.ap()[CH0[q] * P:(CH0[q] + nk) * P, :]
                        .rearrange("(k p) n -> p k n", p=P),
                )
                return tile_

            def _w_chunk(e, q):
                nk = CHUNKS[q]
                tile_ = wpool.tile([P, nk * C], f16, tag=f"wq{q}", name=f"w{e}_{q}")
                nc.sync.dma_start(
                    out=tile_[:].rearrange("p (k c) -> p k c", k=nk),
                    in_=w.ap()[e, CH0[q] * P:(CH0[q] + nk) * P, :]
                        .rearrange("(k p) c -> p k c", p=P),
                )
                return tile_

            # consumption-order issue: W(e0,q0), x(q0), W(e0,q1), x(q1), ...
            w0_q, x_q = [], []
            for q in range(NCH):
                w0_q.append(_w_chunk(0, q))
                x_q.append(_x_chunk(q))

            def _drain(ps, sse, t):
                sq = spool.tile([P, C], f32, tag="sq", name="sq")
                nc.scalar.activation(sq[:], ps[:], mybir.ActivationFunctionType.Square)
                red8 = spool.tile([P, 8], f32, tag="red8", name="red8")
                nc.vector.tensor_reduce(
                    red8[:], sq[:].rearrange("p (g c) -> p g c", g=8),
                    axis=mybir.AxisListType.X, op=mybir.AluOpType.add,
                )
                nc.vector.tensor_reduce(
                    sse[:, t:t + 1], red8[:],
                    axis=mybir.AxisListType.X, op=mybir.AluOpType.add,
                )

            # ---- matmul phase: per expert, k-outer over all 8 token-tile PSUM
            # banks, so k-chunk q is only needed after (q-1) chunks of PE work
            # and the single in-order DMA stream always runs ahead of the PE.
            # The last expert runs t-outer instead (its data is resident by
            # then), so the PSUM drain overlaps the matmuls and the kernel
            # doesn't end with 8 serial square+reduce passes. Each expert's
            # sum-of-squares row DMAs out as soon as that expert finishes.
            for e in range(E):
                w_q = w0_q if e == 0 else [_w_chunk(e, q) for q in range(NCH)]
                sse = cpool.tile([P, NT], f32, tag=f"ssE{e}", name=f"ssE{e}")
                if e < E - 1:
                    ps_t = [psum.tile([P, C], f32, space="PSUM", tag="ps",
                                      name=f"ps{e}_{t}")
                            for t in range(NT)]
                    for k in range(KT):
                        q = max(i for i in range(NCH) if CH0[i] <= k)
                        kq = k - CH0[q]
                        for t in range(NT):
                            nc.tensor.matmul(
                                ps_t[t][:],
                                lhsT=x_q[q][:, kq * TOK + t * P: kq * TOK + (t + 1) * P],
                                rhs=w_q[q][:, kq * C:(kq + 1) * C],
                                start=(k == 0),
                                stop=(k == KT - 1),
                            )
                    for t in range(NT):
                        _drain(ps_t[t], sse, t)
                else:
                    for t in range(NT):
                        ps = psum.tile([P, C], f32, space="PSUM", tag="ps",
                                       name=f"ps{e}_{t}")
                        for k in range(KT):
                            q = max(i for i in range(NCH) if CH0[i] <= k)
                            kq = k - CH0[q]
                            nc.tensor.matmul(
                                ps[:],
                                lhsT=x_q[q][:, kq * TOK + t * P: kq * TOK + (t + 1) * P],
                                rhs=w_q[q][:, kq * C:(kq + 1) * C],
                                start=(k == 0),
                                stop=(k == KT - 1),
                            )
                        _drain(ps, sse, t)
                # partition-major DRAM layout: each partition writes one
                # contiguous 32B run (128 descriptors), not 8 scattered 4B
                # writes (1024 descriptors, ~11us drain tail)
                nc.sync.dma_start(
                    out=ss_out.ap()[e, :].rearrange("(p t) -> p t", p=P),
                    in_=sse[:])
    nc.compile()
    return nc


def _build_phase_b(cap=CAP):
    """Scatter expansion, element-pair form. The dense [2,TOK,E,cap] output
    per core has <=2 nonzero (dispatch, combine) element pairs per token, at
    rows the host fully precomputes: row = (token*E + e_k)*cap + slot_k of a
    [TOK*E*cap, 2] view whose last axis is (dispatch, combine). The device
    just indirect-scatters 8-byte (1.0, prob) pairs -- 16 serial ~1.1us Q7
    descriptor emissions and ~16KB of writes, instead of building/writing
    10.5MB of one-hot rows (scattered-row HBM writes measured only
    ~130GB/s, which made the row variant ~80us+). Capacity-dropped tokens
    get an out-of-range row index and are skipped via bounds_check."""
    import concourse.bass as bass
    i32 = mybir.dt.int32
    NR = 2 * TOK          # (token x choice) scatter elements per core
    NG = NR // P          # 16 scatter groups of 128
    NSPLIT = 4            # independent output tensors to break the WAW chain
    ROWS = TOK * E * cap
    nc = bacc.Bacc("TRN2", target_bir_lowering=False, debug=False, num_devices=NCORES)
    vals = nc.dram_tensor("vals", [NR, 2], f32, kind="ExternalInput")   # (1.0, prob)
    ro = nc.dram_tensor("ro", [NR, 1], i32, kind="ExternalInput")       # target rows
    # Tile serializes same-tensor indirect writes on the completion
    # semaphore (~2.5us per call); round-robining the groups over NSPLIT
    # disjoint tensors (host sums them) leaves only the ~1.1us Q7
    # descriptor emission serial.
    dcs = [nc.dram_tensor(f"dc{j}", [ROWS, 2], f32, kind="ExternalOutput")
           for j in range(NSPLIT)]

    with TileContext(nc) as tc:
        with tc.tile_pool(name="const", bufs=1) as cpool:
            # dummy all-OOB scatter issued before the input DMAs land: warms
            # the SWDGE/indirect path off the critical path
            dum = cpool.tile([P, 2], i32, tag="dum")
            nc.gpsimd.iota(dum[:], pattern=[[1, 2]], base=(1 << 30),
                           channel_multiplier=0)
            nc.gpsimd.indirect_dma_start(
                out=dcs[0].ap()[:, :],
                out_offset=bass.IndirectOffsetOnAxis(ap=dum[:, 0:1], axis=0),
                in_=dum[:, :], in_offset=None,
                bounds_check=ROWS - 1, oob_is_err=False)
            val = cpool.tile([P, NG * 2], f32, tag="val")
            nc.sync.dma_start(out=val[:].rearrange("p (g t) -> p g t", t=2),
                              in_=vals.ap().rearrange("(g p) t -> p g t", p=P))
            rot = cpool.tile([P, NG], i32, tag="rot")
            nc.scalar.dma_start(out=rot[:], in_=ro.ap()[:, 0].rearrange("(g p) -> p g", p=P))
            for g in range(NG):
                nc.gpsimd.indirect_dma_start(
                    out=dcs[g % NSPLIT].ap()[:, :],
                    out_offset=bass.IndirectOffsetOnAxis(ap=rot[:, g:g + 1], axis=0),
                    in_=val[:, 2 * g:2 * g + 2], in_offset=None,
                    bounds_check=ROWS - 1, oob_is_err=False)
    nc.compile()
    return nc


def _get(name, builder):
    if name not in _cache:
        _cache[name] = builder()
    return _cache[name]


def kernel(token_inputs, bottleneck_weights, expert_capacity):
    x = np.ascontiguousarray(np.asarray(token_inputs, dtype=np.float32)).reshape(B * N, D)
    w = np.ascontiguousarray(np.asarray(bottleneck_weights, dtype=np.float32))
    cap = int(expert_capacity)
    assert cap > 0

    w16 = (w * WSCALE).astype(np.float16)
    core_ids = list(range(NCORES))
    in_maps_a = []
    for c in core_ids:
        shard_t = np.ascontiguousarray(x[c * TOK:(c + 1) * TOK].T)   # [2048, 1024]
        in_maps_a.append({"xT": shard_t.astype(np.float16), "w": w16})

    global LAST_IN_MAPS_A, LAST_IN_MAPS_B
    LAST_IN_MAPS_A = in_maps_a
    nc_a = _get("a", _build_phase_a)
    res_a = run_bass_kernel_spmd(nc_a, in_maps_a, core_ids)

    # ---- host glue: logits -> decisions (with fp64 refinement of near-ties),
    # exact capacity cumsums, cross-core offsets, phase-B scatter tables.
    # device ss layout is [E, P*NT] partition-major: element (e, p*NT + t)
    # holds token t*P + p
    ss = np.concatenate(
        [np.asarray(res_a.results[c]["ss"], np.float64)
           .reshape(E, P, NT).transpose(2, 1, 0).reshape(TOK, E)
         for c in core_ids], axis=0) / (WSCALE * WSCALE)
    l = np.sqrt(ss)                                # [8192, E] fp16-pass logits

    order = np.argsort(-l, axis=1, kind="stable")
    l_srt = np.take_along_axis(l, order, 1)
    margin = np.minimum(l_srt[:, 0] - l_srt[:, 1], l_srt[:, 1] - l_srt[:, 2])
    flagged = np.where(margin < THETA)[0]

    if flagged.size:
        # exact logits for the ambiguous tokens (fp64 ~ exact at this scale)
        xf = x[flagged].astype(np.float64)
        wf = np.ascontiguousarray(w.astype(np.float64).transpose(1, 0, 2)).reshape(D, E * C)
        af = (xf @ wf).reshape(-1, E, C)
        l[flagged] = np.sqrt((af * af).sum(-1))
        # re-rank flagged tokens; near-exact ties (below the fp32 resolution of
        # the reference) go to the lower expert index, matching top_k on probs
        for i in flagged:
            li = l[i]
            o = np.argsort(-li, kind="stable")
            for _ in range(E):
                moved = False
                for j in range(E - 1):
                    if (li[o[j]] - li[o[j + 1]] < TIE_TOL) and o[j] > o[j + 1]:
                        o[j], o[j + 1] = o[j + 1], o[j]
                        moved = True
                if not moved:
                    break
            order[i] = o

    e0 = order[:, 0]
    e1 = order[:, 1]
    ex = np.exp(l - l.max(axis=1, keepdims=True))
    probs = ex / ex.sum(axis=1, keepdims=True)
    arN = np.arange(B * N)
    p0 = probs[arN, e0].astype(np.float32)
    p1 = probs[arN, e1].astype(np.float32)

    # exact integer priorities, replicating the reference's k-major cumsum
    slot0 = np.empty(B * N, np.int64)
    slot1 = np.empty(B * N, np.int64)
    arn = np.arange(N)
    for b in range(B):
        sl_ = slice(b * N, (b + 1) * N)
        idx = np.concatenate([e0[sl_], e1[sl_]])
        oh = (idx[:, None] == np.arange(E)[None, :]).astype(np.int64)
        pri = np.cumsum(oh, axis=0) * oh - 1
        slot0[sl_] = pri[:N][arn, e0[sl_]]
        slot1[sl_] = pri[N:][arn, e1[sl_]]

    ar = np.arange(TOK)
    OOB = np.int64(1) << 30
    in_maps_b = []
    for c in core_ids:
        sl_ = slice(c * TOK, (c + 1) * TOK)
        slots = np.concatenate([slot0[sl_], slot1[sl_]])
        ridx = np.concatenate([ar * E + e0[sl_], ar * E + e1[sl_]])
        rows = np.where(slots < cap, ridx * cap + slots, OOB).astype(np.int32)
        vals = np.empty((2 * TOK, 2), np.float32)
        vals[:, 0] = 1.0
        vals[:, 1] = np.concatenate([p0[sl_], p1[sl_]])
        in_maps_b.append({"vals": vals, "ro": rows[:, None]})

    LAST_IN_MAPS_B = in_maps_b
    nc_b = _get(f"b{cap}", lambda: _build_phase_b(cap))
    res_b = run_bass_kernel_spmd(nc_b, in_maps_b, core_ids)

    out = np.empty((2, B, N, E, cap), np.float32)
    for c in core_ids:
        b, h = c // 2, c % 2
        sl_ = slice(h * TOK, (h + 1) * TOK)
        r = res_b.results[c]
        dcr = (r["dc0"] + r["dc1"] + r["dc2"] + r["dc3"]).reshape(TOK, E, cap, 2)
        out[0, b, sl_] = dcr[..., 0]
        out[1, b, sl_] = dcr[..., 1]
    return out
